# revision 71
# baseline (speedup 1.0000x reference)
"""Trainium2 Bass kernel for DigitConvolutionalModel.

Reference computation (B = 32768):
    x: [B, 784] -> reshape [B, 28, 28]
    conv 3x3 valid with w_conv -> [B, 26, 26] -> [B, 676]
    h1 = relu(conv @ W1 + b1)    W1: [676, 100]
    h2 = relu(h1 @ W2 + b2)      W2: [100, 100]
    out = h2 @ W3 + b3           W3: [100, 10]

Strategy
--------
Pure data parallel: batch split 8 ways (4096 rows/core), weights replicated.
The conv is linear, so it is folded into W1 on the host:
    conv(x) @ W1 == x @ (M @ W1) = x @ W1e,  W1e: [784, 100]
removing the conv from the device entirely (exact up to fp rounding).

On-device layout is "transposed": features on SBUF partitions, batch on the
free dimension, so each layer's PSUM output feeds the next matmul directly
as the moving operand. The host pre-transposes x per core and lays it out
as [128, 6, B_LOC] (contraction split 784 = 6*128 + 16; the 16-row tail is
a separate [16, B_LOC] resident tile) so every x DMA uses all 128
partitions with long contiguous runs.

x and the weights are cast to fp16 on the host: fp16's 10-bit mantissa
keeps end-to-end error at the TF32 level (~1e-3 absmax vs the fp32
reference, measured) while halving HBM traffic and running every matmul at
full PE rate. The kernel is HBM-bandwidth bound streaming x (~6.4 MB/core).
x streams in per-chunk tiles from a small pool whose buffer count
throttles in-flight DMAs so chunks complete in consumption order; each
group's relu/layer-2/layer-3 epilogue is emitted into the next group's
matmul stream so the DMA slot-release chain never runs through the slow
ACT/DVE stages, and the 16-row tail matmul accumulates early so only one
matmul sits on the exposed critical path after the last chunk lands.
"""

import numpy as np

N_CORES = 8
B = 32768
B_LOC = B // N_CORES          # 4096 rows per core
NT = 512                      # matmul moving-dim tile (PSUM bank limit)
# batch columns per chunk group; the last group is small so the epilogue
# that runs after the x stream ends is short
GROUPS = [2048, 1024, 512, 512]
KC = 6                        # full 128-row contraction chunks
KT = 784 - KC * 128           # 16-row tail
H = 100                       # hidden width
O = 10                        # output width
XBUFS = 14                    # in-flight x chunk DMAs
WARMUP_MMS = 0                # dummy matmuls to warm the PE clock gate
N_PS1 = 5                     # rotating layer-1 PSUM accumulator banks

_COMPILED = {}
LAST_RESULTS = None


def _build_nc():
    import concourse.mybir as mybir
    from concourse import bacc
    from concourse.tile import TileContext, add_dep_helper

    f32 = mybir.dt.float32
    f16 = mybir.dt.float16

    nc = bacc.Bacc(
        "TRN2", target_bir_lowering=False, debug=False, num_devices=N_CORES
    )
    xt = nc.dram_tensor("xt", [128, KC, B_LOC], f16, kind="ExternalInput")
    w1 = nc.dram_tensor("w1", [128, KC, H], f16, kind="ExternalInput")
    # packed [16, 100 + B_LOC]: W1e tail rows | x tail rows
    wxl = nc.dram_tensor("wxl", [KT, H + B_LOC], f16, kind="ExternalInput")
    # packed [100, 110]: W2 | W3
    w23 = nc.dram_tensor("w23", [H, H + O], f16, kind="ExternalInput")
    # packed [100, 3]: b1 | b2 | b3 (b3 on partitions 0..9)
    bb = nc.dram_tensor("bb", [H, 3], f32, kind="ExternalInput")
    ot = nc.dram_tensor("ot", [O, B_LOC], f32, kind="ExternalOutput")

    relu = mybir.ActivationFunctionType.Relu
    add = mybir.AluOpType.add
    amax = mybir.AluOpType.max

    with TileContext(nc) as tc:
        with (
            tc.tile_pool(name="wpool", bufs=1) as wpool,
            tc.tile_pool(name="xpool", bufs=XBUFS) as xpool,
            tc.tile_pool(name="hpool", bufs=3) as hpool,
            tc.tile_pool(name="opool", bufs=3) as opool,
            tc.tile_pool(name="ppool", bufs=1, space="PSUM") as ppool,
        ):
            # W1 loads first on the scalar HWDGE ring (the first matmul
            # needs it); x streams on the sync ring. The remaining weights
            # are not needed until the first epilogue (~15 us in), so their
            # DMAs are held back behind early x chunks to keep startup
            # bandwidth on the critical path.
            w1_t = wpool.tile([128, KC, H], f16)
            nc.sync.dma_start(out=w1_t, in_=w1.ap())
            wxl_t = wpool.tile([KT, H + B_LOC], f16)
            nc.scalar.dma_start(out=wxl_t, in_=wxl.ap())
            w1l_t = wxl_t[:, 0:H]
            xl_t = wxl_t[:, H : H + B_LOC]
            w23_t = wpool.tile([H, H + O], f16)
            nc.scalar.dma_start(out=w23_t, in_=w23.ap())
            bb_t = wpool.tile([H, 3], f32)
            nc.scalar.dma_start(out=bb_t, in_=bb.ap())

            w2_t = w23_t[:, 0:H]
            w3_t = w23_t[:, H : H + O]
            b1_t = bb_t[:, 0:1]
            b2_t = bb_t[:, 1:2]
            b3_t = bb_t[:O, 2:3]

            def epilogue_last(g0, ps1):
                # the final epilogue is fully exposed after the x stream
                # ends; run it in two 256-column half-chains so the
                # ACT/PE/DVE stages pipeline and the serial latency halves
                NH = NT // 2
                nc.tensor.matmul(
                    ps1[:H, :],
                    lhsT=w1l_t,
                    rhs=xl_t[:, g0 : g0 + NT],
                    start=False,
                    stop=True,
                )
                h1 = hpool.tile([H, NT], f16, tag="h1", bufs=4, name="h1_l")
                h2 = hpool.tile([H, NT], f16, tag="h2", bufs=4, name="h2_l")
                o_t = opool.tile([O, NT], f32, tag="o_t", bufs=4, name="o_l")
                ps2 = ppool.tile([128, NT], f32, tag="ps2", bufs=2, name="ps2")
                ps3 = ppool.tile([128, NT], f32, tag="ps3", bufs=1, name="ps3")
                for hh in range(2):
                    cs = slice(hh * NH, (hh + 1) * NH)
                    nc.scalar.activation(h1[:, cs], ps1[:H, cs], relu, bias=b1_t)
                    nc.tensor.matmul(
                        ps2[:H, cs], lhsT=w2_t, rhs=h1[:, cs],
                        start=True, stop=True,
                    )
                    nc.vector.tensor_scalar(
                        h2[:, cs], ps2[:H, cs], b2_t, 0.0, add, amax
                    )
                    nc.tensor.matmul(
                        ps3[:O, cs], lhsT=w3_t, rhs=h2[:, cs],
                        start=True, stop=True,
                    )
                    nc.scalar.add(o_t[:, cs], ps3[:O, cs], b3_t)
                nc.gpsimd.dma_start(out=ot.ap()[:, g0 : g0 + NT], in_=o_t)

            def epilogue(g0, subt, ps1s):
                # stage-major across subtiles so the per-engine FIFOs don't
                # head-of-line block the chains
                h1s, h2s, o_ts = [], [], []
                for s in range(subt):
                    h1 = hpool.tile([H, NT], f16, tag="h1", bufs=4, name=f"h1_{s}")
                    nc.scalar.activation(h1, ps1s[s][:H, :], relu, bias=b1_t)
                    h1s.append(h1)
                for s in range(subt):
                    ps2 = ppool.tile([128, NT], f32, tag="ps2", bufs=2, name="ps2")
                    nc.tensor.matmul(
                        ps2[:H, :], lhsT=w2_t, rhs=h1s[s], start=True, stop=True
                    )
                    h2 = hpool.tile([H, NT], f16, tag="h2", bufs=4, name=f"h2_{s}")
                    nc.vector.tensor_scalar(h2, ps2[:H, :], b2_t, 0.0, add, amax)
                    h2s.append(h2)
                for s in range(subt):
                    ps3 = ppool.tile([128, NT], f32, tag="ps3", bufs=1, name="ps3")
                    nc.tensor.matmul(
                        ps3[:O, :], lhsT=w3_t, rhs=h2s[s], start=True, stop=True
                    )
                    o_t = opool.tile([O, NT], f32, tag="o_t", bufs=4, name=f"o_{s}")
                    nc.scalar.add(o_t, ps3[:O, :], b3_t)
                    o_ts.append(o_t)
                for s in range(subt):
                    n0 = g0 + s * NT
                    nc.gpsimd.dma_start(
                        out=ot.ap()[:, n0 : n0 + NT], in_=o_ts[s]
                    )

            # software pipeline: group g's epilogue is emitted two chunks
            # into group g+1's mm1 stream — late enough that its ACT/DVE
            # inputs are ready when the PE reaches it, early enough that
            # the x-chunk slot releases gating the DMA stream don't chain
            # through the whole epilogue.
            pending = None  # (g0, subt, ps1s)
            ps1_rot = 0
            g0 = 0
            for g, ntd in enumerate(GROUPS):
                gs = slice(g0, g0 + ntd)
                subt = ntd // NT
                xc = []
                for c in range(KC):
                    x_c = xpool.tile([128, ntd], f16, tag="xc", name=f"xc{c}")
                    nc.sync.dma_start(out=x_c, in_=xt.ap()[:, c, gs])
                    xc.append(x_c)

                ps1s = [
                    ppool.tile(
                        [128, NT],
                        f32,
                        tag=f"ps1_{(ps1_rot + s) % N_PS1}",
                        bufs=1,
                        name=f"ps1_{s}",
                    )
                    for s in range(subt)
                ]
                ps1_rot += subt
                # each arriving chunk immediately feeds all subtiles'
                # accumulating matmuls
                for c in range(KC):
                    for s in range(subt):
                        nc.tensor.matmul(
                            ps1s[s][:H, :],
                            lhsT=w1_t[:, c, :],
                            rhs=xc[c][:, s * NT : (s + 1) * NT],
                            start=(c == 0),
                            stop=(c == KC - 1),
                        )
                    if c == 0:
                        for s in range(subt):
                            nc.tensor.matmul(
                                ps1s[s][:H, :],
                                lhsT=w1l_t,
                                rhs=xl_t[:, g0 + s * NT : g0 + (s + 1) * NT],
                                start=False,
                                stop=False,
                            )
                    if c == 0 and pending is not None:
                        epilogue(*pending)
                        pending = None
                pending = (g0, subt, ps1s)
                g0 += ntd
            epilogue(*pending)

    nc.finalize()
    return nc


def _fold_conv_into_w1(w_conv, W1):
    """W1e[784, 100] such that x @ W1e == conv3x3(x) @ W1 (exact linear fold)."""
    W1e = np.zeros((28, 28, H), np.float64)
    W1r = W1.astype(np.float64).reshape(26, 26, H)
    wc = w_conv.astype(np.float64)
    for di in range(3):
        for dj in range(3):
            W1e[di : di + 26, dj : dj + 26, :] += wc[di, dj] * W1r
    return W1e.reshape(784, H).astype(np.float32)


def kernel(x, w_conv, W1, b1, W2, b2, W3, b3):
    from concourse.bass_utils import run_bass_kernel_spmd

    global LAST_RESULTS

    x = np.asarray(x, np.float32)
    W1e = _fold_conv_into_w1(np.asarray(w_conv), np.asarray(W1))
    # [784, 100]: rows 0..767 -> [128, KC, 100]; rows 768..783 -> [16, 100]
    w1_dev = np.ascontiguousarray(
        W1e[: KC * 128].reshape(KC, 128, H).transpose(1, 0, 2)
    ).astype(np.float16)
    w1l_dev = W1e[KC * 128 :].astype(np.float16)      # [16, 100]
    w23_dev = np.zeros((H, H + O), np.float16)
    w23_dev[:, 0:H] = np.asarray(W2, np.float32).astype(np.float16)
    w23_dev[:, H : H + O] = np.asarray(W3, np.float32).astype(np.float16)
    bb_dev = np.zeros((H, 3), np.float32)
    bb_dev[:, 0] = np.asarray(b1, np.float32)
    bb_dev[:, 1] = np.asarray(b2, np.float32)
    bb_dev[:O, 2] = np.asarray(b3, np.float32)

    in_maps = []
    for c in range(N_CORES):
        xs = x[c * B_LOC : (c + 1) * B_LOC]          # [B_LOC, 784]
        xT = xs.T.astype(np.float16)                  # [784, B_LOC] fp16
        # main: [128, KC, B_LOC], element [p, k, n] = xT[k*128 + p, n]
        xmain = np.ascontiguousarray(
            xT[: KC * 128].reshape(KC, 128, B_LOC).transpose(1, 0, 2)
        )
        wxl_dev = np.concatenate([w1l_dev, xT[KC * 128 :]], axis=1)
        in_maps.append(
            {
                "xt": xmain,
                "wxl": np.ascontiguousarray(wxl_dev),
                "w1": w1_dev,
                "w23": w23_dev,
                "bb": bb_dev,
            }
        )

    if "nc" not in _COMPILED:
        _COMPILED["nc"] = _build_nc()
    nc = _COMPILED["nc"]

    res = run_bass_kernel_spmd(nc, in_maps, core_ids=list(range(N_CORES)))
    LAST_RESULTS = res

    out = np.empty((B, O), np.float32)
    for c in range(N_CORES):
        out[c * B_LOC : (c + 1) * B_LOC] = res.results[c]["ot"].T
    return out


# revision 72
# speedup vs baseline: 1.0240x; 1.0240x over previous
"""Trainium2 Bass kernel for DigitConvolutionalModel.

Reference computation (B = 32768):
    x: [B, 784] -> reshape [B, 28, 28]
    conv 3x3 valid with w_conv -> [B, 26, 26] -> [B, 676]
    h1 = relu(conv @ W1 + b1)    W1: [676, 100]
    h2 = relu(h1 @ W2 + b2)      W2: [100, 100]
    out = h2 @ W3 + b3           W3: [100, 10]

Strategy
--------
Pure data parallel: batch split 8 ways (4096 rows/core), weights replicated.
The conv is linear, so it is folded into W1 on the host:
    conv(x) @ W1 == x @ (M @ W1) = x @ W1e,  W1e: [784, 100]
removing the conv from the device entirely (exact up to fp rounding).

On-device layout is "transposed": features on SBUF partitions, batch on the
free dimension, so each layer's PSUM output feeds the next matmul directly
as the moving operand. The host pre-transposes x per core and lays it out
as [128, 6, B_LOC] (contraction split 784 = 6*128 + 16; the 16-row tail is
a separate [16, B_LOC] resident tile) so every x DMA uses all 128
partitions with long contiguous runs.

x and the weights are cast to fp16 on the host: fp16's 10-bit mantissa
keeps end-to-end error at the TF32 level (~1e-3 absmax vs the fp32
reference, measured) while halving HBM traffic and running every matmul at
full PE rate. The kernel is HBM-bandwidth bound streaming x (~6.4 MB/core).
x streams in per-chunk tiles from a small pool whose buffer count
throttles in-flight DMAs so chunks complete in consumption order; each
group's relu/layer-2/layer-3 epilogue is emitted into the next group's
matmul stream so the DMA slot-release chain never runs through the slow
ACT/DVE stages, and the 16-row tail matmul accumulates early so only one
matmul sits on the exposed critical path after the last chunk lands.
"""

import numpy as np

N_CORES = 8
B = 32768
B_LOC = B // N_CORES          # 4096 rows per core
NT = 512                      # matmul moving-dim tile (PSUM bank limit)
# batch columns per chunk group; the last group is small so the epilogue
# that runs after the x stream ends is short
GROUPS = [2048, 1024, 512, 512]
KC = 6                        # full 128-row contraction chunks
KT = 784 - KC * 128           # 16-row tail
H = 100                       # hidden width
O = 10                        # output width
XBUFS = 14                    # in-flight x chunk DMAs
WARMUP_MMS = 0                # dummy matmuls to warm the PE clock gate
N_PS1 = 5                     # rotating layer-1 PSUM accumulator banks

_COMPILED = {}
LAST_RESULTS = None


def _build_nc():
    import concourse.mybir as mybir
    from concourse import bacc
    from concourse.tile import TileContext, add_dep_helper

    f32 = mybir.dt.float32
    f16 = mybir.dt.float16

    nc = bacc.Bacc(
        "TRN2", target_bir_lowering=False, debug=False, num_devices=N_CORES
    )
    xt = nc.dram_tensor("xt", [128, KC, B_LOC], f16, kind="ExternalInput")
    w1 = nc.dram_tensor("w1", [128, KC, H], f16, kind="ExternalInput")
    # packed [16, 100 + B_LOC]: W1e tail rows | x tail rows
    wxl = nc.dram_tensor("wxl", [KT, H + B_LOC], f16, kind="ExternalInput")
    # packed [100, 110]: W2 | W3
    w23 = nc.dram_tensor("w23", [H, H + O], f16, kind="ExternalInput")
    # packed [100, 3]: b1 | b2 | b3 (b3 on partitions 0..9)
    bb = nc.dram_tensor("bb", [H, 3], f32, kind="ExternalInput")
    ot = nc.dram_tensor("ot", [O, B_LOC], f32, kind="ExternalOutput")

    relu = mybir.ActivationFunctionType.Relu
    add = mybir.AluOpType.add
    amax = mybir.AluOpType.max

    with TileContext(nc) as tc:
        with (
            tc.tile_pool(name="wpool", bufs=1) as wpool,
            tc.tile_pool(name="xpool", bufs=XBUFS) as xpool,
            tc.tile_pool(name="hpool", bufs=3) as hpool,
            tc.tile_pool(name="opool", bufs=3) as opool,
            tc.tile_pool(name="ppool", bufs=1, space="PSUM") as ppool,
        ):
            # W1 loads first on the scalar HWDGE ring (the first matmul
            # needs it); x streams on the sync ring. The remaining weights
            # are not needed until the first epilogue (~15 us in), so their
            # DMAs are held back behind early x chunks to keep startup
            # bandwidth on the critical path.
            w1_t = wpool.tile([128, KC, H], f16)
            nc.sync.dma_start(out=w1_t, in_=w1.ap())
            wxl_t = wpool.tile([KT, H + B_LOC], f16)
            nc.scalar.dma_start(out=wxl_t, in_=wxl.ap())
            w1l_t = wxl_t[:, 0:H]
            xl_t = wxl_t[:, H : H + B_LOC]
            w23_t = wpool.tile([H, H + O], f16)
            nc.scalar.dma_start(out=w23_t, in_=w23.ap())
            bb_t = wpool.tile([H, 3], f32)
            nc.scalar.dma_start(out=bb_t, in_=bb.ap())

            w2_t = w23_t[:, 0:H]
            w3_t = w23_t[:, H : H + O]
            b1_t = bb_t[:, 0:1]
            b2_t = bb_t[:, 1:2]
            b3_t = bb_t[:O, 2:3]

            def epilogue_last(g0, ps1):
                # the final epilogue is fully exposed after the x stream
                # ends; run it in two 256-column half-chains so the
                # ACT/PE/DVE stages pipeline and the serial latency halves
                NH = NT // 2
                nc.tensor.matmul(
                    ps1[:H, :],
                    lhsT=w1l_t,
                    rhs=xl_t[:, g0 : g0 + NT],
                    start=False,
                    stop=True,
                )
                h1 = hpool.tile([H, NT], f16, tag="h1", bufs=4, name="h1_l")
                h2 = hpool.tile([H, NT], f16, tag="h2", bufs=4, name="h2_l")
                o_t = opool.tile([O, NT], f32, tag="o_t", bufs=4, name="o_l")
                ps2 = ppool.tile([128, NT], f32, tag="ps2", bufs=2, name="ps2")
                ps3 = ppool.tile([128, NT], f32, tag="ps3", bufs=1, name="ps3")
                for hh in range(2):
                    cs = slice(hh * NH, (hh + 1) * NH)
                    nc.scalar.activation(h1[:, cs], ps1[:H, cs], relu, bias=b1_t)
                    nc.tensor.matmul(
                        ps2[:H, cs], lhsT=w2_t, rhs=h1[:, cs],
                        start=True, stop=True,
                    )
                    nc.vector.tensor_scalar(
                        h2[:, cs], ps2[:H, cs], b2_t, 0.0, add, amax
                    )
                    nc.tensor.matmul(
                        ps3[:O, cs], lhsT=w3_t, rhs=h2[:, cs],
                        start=True, stop=True,
                    )
                    nc.scalar.add(o_t[:, cs], ps3[:O, cs], b3_t)
                nc.gpsimd.dma_start(out=ot.ap()[:, g0 : g0 + NT], in_=o_t)

            def epilogue(g0, subt, ps1s):
                # stage-major across subtiles so the per-engine FIFOs don't
                # head-of-line block the chains
                h1s, h2s, o_ts = [], [], []
                for s in range(subt):
                    h1 = hpool.tile([H, NT], f16, tag="h1", bufs=4, name=f"h1_{s}")
                    nc.scalar.activation(h1, ps1s[s][:H, :], relu, bias=b1_t)
                    h1s.append(h1)
                for s in range(subt):
                    ps2 = ppool.tile([128, NT], f32, tag="ps2", bufs=2, name="ps2")
                    nc.tensor.matmul(
                        ps2[:H, :], lhsT=w2_t, rhs=h1s[s], start=True, stop=True
                    )
                    h2 = hpool.tile([H, NT], f16, tag="h2", bufs=4, name=f"h2_{s}")
                    nc.vector.tensor_scalar(h2, ps2[:H, :], b2_t, 0.0, add, amax)
                    h2s.append(h2)
                for s in range(subt):
                    ps3 = ppool.tile([128, NT], f32, tag="ps3", bufs=1, name="ps3")
                    nc.tensor.matmul(
                        ps3[:O, :], lhsT=w3_t, rhs=h2s[s], start=True, stop=True
                    )
                    o_t = opool.tile([O, NT], f32, tag="o_t", bufs=4, name=f"o_{s}")
                    nc.scalar.add(o_t, ps3[:O, :], b3_t)
                    o_ts.append(o_t)
                for s in range(subt):
                    n0 = g0 + s * NT
                    nc.gpsimd.dma_start(
                        out=ot.ap()[:, n0 : n0 + NT], in_=o_ts[s]
                    )

            # software pipeline: group g's epilogue is emitted two chunks
            # into group g+1's mm1 stream — late enough that its ACT/DVE
            # inputs are ready when the PE reaches it, early enough that
            # the x-chunk slot releases gating the DMA stream don't chain
            # through the whole epilogue.
            pending = None  # (g0, subt, ps1s)
            ps1_rot = 0
            g0 = 0
            for g, ntd in enumerate(GROUPS):
                gs = slice(g0, g0 + ntd)
                subt = ntd // NT
                xc = []
                for c in range(KC):
                    x_c = xpool.tile([128, ntd], f16, tag="xc", name=f"xc{c}")
                    nc.sync.dma_start(out=x_c, in_=xt.ap()[:, c, gs])
                    xc.append(x_c)

                ps1s = [
                    ppool.tile(
                        [128, NT],
                        f32,
                        tag=f"ps1_{(ps1_rot + s) % N_PS1}",
                        bufs=1,
                        name=f"ps1_{s}",
                    )
                    for s in range(subt)
                ]
                ps1_rot += subt
                # each arriving chunk immediately feeds all subtiles'
                # accumulating matmuls
                for c in range(KC):
                    for s in range(subt):
                        nc.tensor.matmul(
                            ps1s[s][:H, :],
                            lhsT=w1_t[:, c, :],
                            rhs=xc[c][:, s * NT : (s + 1) * NT],
                            start=(c == 0),
                            stop=(c == KC - 1),
                        )
                    if c == 2:
                        for s in range(subt):
                            nc.tensor.matmul(
                                ps1s[s][:H, :],
                                lhsT=w1l_t,
                                rhs=xl_t[:, g0 + s * NT : g0 + (s + 1) * NT],
                                start=False,
                                stop=False,
                            )
                    if c == 0 and pending is not None:
                        epilogue(*pending)
                        pending = None
                pending = (g0, subt, ps1s)
                g0 += ntd
            epilogue(*pending)

    nc.finalize()
    return nc


def _fold_conv_into_w1(w_conv, W1):
    """W1e[784, 100] such that x @ W1e == conv3x3(x) @ W1 (exact linear fold)."""
    W1e = np.zeros((28, 28, H), np.float64)
    W1r = W1.astype(np.float64).reshape(26, 26, H)
    wc = w_conv.astype(np.float64)
    for di in range(3):
        for dj in range(3):
            W1e[di : di + 26, dj : dj + 26, :] += wc[di, dj] * W1r
    return W1e.reshape(784, H).astype(np.float32)


def kernel(x, w_conv, W1, b1, W2, b2, W3, b3):
    from concourse.bass_utils import run_bass_kernel_spmd

    global LAST_RESULTS

    x = np.asarray(x, np.float32)
    W1e = _fold_conv_into_w1(np.asarray(w_conv), np.asarray(W1))
    # [784, 100]: rows 0..767 -> [128, KC, 100]; rows 768..783 -> [16, 100]
    w1_dev = np.ascontiguousarray(
        W1e[: KC * 128].reshape(KC, 128, H).transpose(1, 0, 2)
    ).astype(np.float16)
    w1l_dev = W1e[KC * 128 :].astype(np.float16)      # [16, 100]
    w23_dev = np.zeros((H, H + O), np.float16)
    w23_dev[:, 0:H] = np.asarray(W2, np.float32).astype(np.float16)
    w23_dev[:, H : H + O] = np.asarray(W3, np.float32).astype(np.float16)
    bb_dev = np.zeros((H, 3), np.float32)
    bb_dev[:, 0] = np.asarray(b1, np.float32)
    bb_dev[:, 1] = np.asarray(b2, np.float32)
    bb_dev[:O, 2] = np.asarray(b3, np.float32)

    in_maps = []
    for c in range(N_CORES):
        xs = x[c * B_LOC : (c + 1) * B_LOC]          # [B_LOC, 784]
        xT = xs.T.astype(np.float16)                  # [784, B_LOC] fp16
        # main: [128, KC, B_LOC], element [p, k, n] = xT[k*128 + p, n]
        xmain = np.ascontiguousarray(
            xT[: KC * 128].reshape(KC, 128, B_LOC).transpose(1, 0, 2)
        )
        wxl_dev = np.concatenate([w1l_dev, xT[KC * 128 :]], axis=1)
        in_maps.append(
            {
                "xt": xmain,
                "wxl": np.ascontiguousarray(wxl_dev),
                "w1": w1_dev,
                "w23": w23_dev,
                "bb": bb_dev,
            }
        )

    if "nc" not in _COMPILED:
        _COMPILED["nc"] = _build_nc()
    nc = _COMPILED["nc"]

    res = run_bass_kernel_spmd(nc, in_maps, core_ids=list(range(N_CORES)))
    LAST_RESULTS = res

    out = np.empty((B, O), np.float32)
    for c in range(N_CORES):
        out[c * B_LOC : (c + 1) * B_LOC] = res.results[c]["ot"].T
    return out
